# revision 3
# baseline (speedup 1.0000x reference)
"""Trainium2 Bass kernel for the autoregressive GRU decoder (v7).

Structure (HW-measured design):
  - pure data parallel over batch: 8 cores x 2048 rows, no collectives;
    weights folded on host (linear y-feedback absorbed into H->H gate
    matrices), fp16 state/weights, fp32 PSUM.
  - per core, batch is split into 2 pair-pipelines (2x 1024) running
    HALF A STEP out of phase, so the in-order per-engine queues
    alternate between the pairs' phases and no convoy stalls form.
  - u = (ghn+b_hn)*r is written by DVE IN-PLACE over ghn in PSUM and
    the in-gate matmul accumulates on top with start=False (kills the
    identity-matmul accumulate, PE 24->20 matmuls/step).
  - m2 = z*h and zbar = 1-z are emitted AFTER u on the DVE queue
    (in-order issue would otherwise stall u behind sigma_z); zbar runs
    in DVE 4x tensor_scalar mode (362ns) — cheaper and lower-variance
    than Pool, whose SBUF port is shared with DVE.
  - y copies on DVE (ACT is the 6-op/step metronome); b_out is folded
    on the host after gather.

v6 + half-step SKEWED pair pipelines: the two batch-pair pipelines are
emitted phase-shifted by half a step (ph2(p0,t), ph1(p0,t+1), ph2(p1,t),
ph1(p1,t+1), ...), so each in-order engine queue alternates between the
two pairs' phases and arrivals spread evenly — no convoy stalls at step
boundaries.

ph1(p,t): r/z-mms, sigmoids, zbar(Pool), hn-mms, u (DVE, in-place over
          ghn in PSUM), m2, in-mms (accumulate over u, start=False).
ph2(p,t): tanh, g, h', y-mms (PE), ycopy.

PSUM bank map as v6:
  b0: gr0 -> ghn0 -> u0+gin0 -> tanh    b1: chunk 1
  b4: gr2 -> ...                        b5: chunk 3
  b2: gz0                               b3: y(p0,t-1) -> gz1
  b6: gz2                               b7: y(p1,t-1) -> gz3

Sharding: pure data parallel over batch, 8 cores x 2048, no collectives.
"""

import os

import numpy as np

B, T, I, H, SEQLEN = 16384, 60, 32, 128, 30
STEPS = T - SEQLEN  # 30
NCORES = 8
BC = B // NCORES  # 2048 batch rows per core
NCH = 4
C = BC // NCH  # 512 = one PSUM bank of fp32

LAST_RESULT = None
HOST_BOUT = True
_CACHE = {}

ZBAR_ENG = os.environ.get("K_ZBAR_ENG", "vector")
M2_ENG = os.environ.get("K_M2_ENG", "vector")
YC = os.environ.get("K_YC", "vector")
DMA_EVERY = int(os.environ.get("K_DMA_EVERY", "1"))
WARMUP_MM = int(os.environ.get("K_WARMUP_MM", "0"))
WARMUP_ONCE = int(os.environ.get("K_WARMUP_ONCE", "0"))
GFUSE = int(os.environ.get("K_GFUSE", "0"))


def _build(repeats=1):
    from contextlib import ExitStack

    import concourse.bacc as bacc
    import concourse.bass as bass
    import concourse.mybir as mybir
    import concourse.tile as tile

    f32 = mybir.dt.float32
    f16 = mybir.dt.float16
    Alu = mybir.AluOpType
    Act = mybir.ActivationFunctionType

    nc = bacc.Bacc()

    CW = 6 * H + I
    dcst = nc.dram_tensor("cst", [H, CW], f16, kind="ExternalInput")
    dw0 = nc.dram_tensor("w0", [I, 3 * H], f16, kind="ExternalInput")
    dx0 = nc.dram_tensor("x0", [I, BC], f16, kind="ExternalInput")
    dh = nc.dram_tensor("h0t", [H, BC], f16, kind="ExternalInput")
    dbias = nc.dram_tensor("bias", [H, 8], f32, kind="ExternalInput")
    dout = nc.dram_tensor("out", [64, STEPS * 2 * C], f32, kind="ExternalOutput")

    with ExitStack() as ctx:
        tc = ctx.enter_context(tile.TileContext(nc))
        const = ctx.enter_context(tc.tile_pool(name="const", bufs=1))
        work = ctx.enter_context(tc.tile_pool(name="work", bufs=1))
        psum = ctx.enter_context(tc.tile_pool(name="psum", bufs=1, space="PSUM"))

        def load_const(dram, shape, name, dtype=None):
            t = const.tile(shape, dtype or dram.dtype, tag=name)
            nc.sync.dma_start(out=t[:], in_=dram[:, :])
            return t

        scst = load_const(dcst, [H, CW], "cst")
        sw0 = load_const(dw0, [I, 3 * H], "w0")
        sx0 = load_const(dx0, [I, BC], "x0")
        sbias = load_const(dbias, [H, 8], "bias")
        hT = load_const(dh, [H, BC], "h")

        A_r = scst[:, 0 * H : 1 * H]
        A_z = scst[:, 1 * H : 2 * H]
        A_hn = scst[:, 2 * H : 3 * H]
        A_in = scst[:, 3 * H : 4 * H]
        A0_r = scst[:, 4 * H : 5 * H]
        A0_z = scst[:, 5 * H : 6 * H]
        WoutT = scst[:, 6 * H : 6 * H + I]
        W0_r = sw0[:, 0 * H : 1 * H]
        W0_z = sw0[:, 1 * H : 2 * H]
        W0_n = sw0[:, 2 * H : 3 * H]

        b_r = sbias[:, 0:1]
        b_z = sbias[:, 1:2]
        b_hn = sbias[:, 2:3]
        b_in = sbias[:, 3:4]
        b0_r = sbias[:, 4:5]
        b0_z = sbias[:, 5:6]
        b0_in = sbias[:, 6:7]
        b_y = sbias[:, 7:8]

        names = ["r", "z", "n", "w", "g", "m2"]
        wt = {}
        for nm in names:
            for par in range(2):
                wt[nm, par] = work.tile(
                    [H, BC], f16, tag=f"{nm}{par}", name=f"{nm}{par}"
                )
        h2 = work.tile([H, BC], f16, tag="h2")
        h_bufs = [hT, h2]
        y_all = work.tile([64, STEPS * 2 * C], f32, tag="y")

        P = psum.tile([128, 8 * C], f32, tag="P", bufs=1)

        RB = [0, 1, 4, 5]
        NB = [2, 3, 6, 7]
        YB = [3, 7]

        def bank(b, parts=slice(0, 128)):
            return P[parts, b * C : (b + 1) * C]

        def pairb(b0):
            return P[:, b0 * C : (b0 + 2) * C]

        def x0mov(c):
            return sx0[:, c * C : (c + 1) * C]

        def half(tile_, p):
            return tile_[:, p * 2 * C : (p + 1) * 2 * C]

        def ph1(p, t):
            """r/z-mms, sigmoids, zbar, hn-mms, u, m2, in-mms for pair p."""
            first = t == 0
            par = t % 2
            hcur = h_bufs[par]
            r_sb, z_sb = wt["r", par], wt["z", par]
            w_sb, m2_sb = wt["w", par], wt["m2", par]

            def hmov(c):
                return hcur[:, c * C : (c + 1) * C]

            for c in (2 * p, 2 * p + 1):
                nc.tensor.matmul(bank(RB[c]), A0_r if first else A_r,
                                 hmov(c), start=True, stop=not first)
                if first:
                    nc.tensor.matmul(bank(RB[c]), W0_r, x0mov(c),
                                     start=False, stop=True)
            for c in (2 * p, 2 * p + 1):
                nc.tensor.matmul(bank(NB[c]), A0_z if first else A_z, hmov(c),
                                 start=True, stop=not first)
                if first:
                    nc.tensor.matmul(bank(NB[c]), W0_z, x0mov(c),
                                     start=False, stop=True)

            cb_r, cb_z = (b0_r, b0_z) if first else (b_r, b_z)
            nc.scalar.activation(half(r_sb, p), pairb(RB[2 * p]),
                                 Act.Sigmoid, bias=cb_r)
            nc.scalar.activation(half(z_sb, p), pairb(NB[2 * p]),
                                 Act.Sigmoid, bias=cb_z)
            if not GFUSE and ZBAR_ENG != "vector":
                getattr(nc, ZBAR_ENG).tensor_scalar(
                    half(w_sb, p), half(z_sb, p), -1.0, 1.0,
                    Alu.mult, Alu.add)

            for c in (2 * p, 2 * p + 1):
                nc.tensor.matmul(bank(RB[c]), A_hn, hmov(c),
                                 start=True, stop=True)

            nc.vector.scalar_tensor_tensor(
                pairb(RB[2 * p]), pairb(RB[2 * p]), b_hn, half(r_sb, p),
                Alu.add, Alu.mult)
            m2e = M2_ENG
            if m2e == "split":
                m2e = "gpsimd" if p == 0 else "vector"
            getattr(nc, m2e).tensor_tensor(
                half(m2_sb, p), half(z_sb, p), half(hcur, p), Alu.mult)
            if not GFUSE and ZBAR_ENG == "vector":
                nc.vector.tensor_scalar(
                    half(w_sb, p), half(z_sb, p), -1.0, 1.0,
                    Alu.mult, Alu.add)
            for c in (2 * p, 2 * p + 1):
                nc.tensor.matmul(bank(RB[c]),
                                 W0_n if first else A_in,
                                 x0mov(c) if first else hmov(c),
                                 start=False, stop=True,
                                 skip_group_check=True)

        def ph2(p, t):
            """tanh, g, h', y-mms, ycopy for pair p."""
            first = t == 0
            par = t % 2
            hcur, hnxt = h_bufs[par], h_bufs[1 - par]
            n_sb = wt["n", par]
            w_sb, g_sb, m2_sb = wt["w", par], wt["g", par], wt["m2", par]

            cb_in = b0_in if first else b_in
            nc.scalar.activation(half(n_sb, p), pairb(RB[2 * p]),
                                 Act.Tanh, bias=cb_in)
            if GFUSE:
                # g' = (z - 1) * n ; h' = m2 - g'  (no zbar op needed)
                nc.vector.scalar_tensor_tensor(
                    half(g_sb, p), half(wt["z", t % 2], p), -1.0,
                    half(n_sb, p), Alu.add, Alu.mult)
                nc.vector.tensor_tensor(half(hnxt, p), half(m2_sb, p),
                                        half(g_sb, p), Alu.subtract)
            else:
                nc.vector.tensor_tensor(half(g_sb, p), half(w_sb, p),
                                        half(n_sb, p), Alu.mult)
                nc.vector.tensor_tensor(half(hnxt, p), half(g_sb, p),
                                        half(m2_sb, p), Alu.add)

            # y(t) for pair p: reads h'(t) = hnxt
            for k in range(2):
                c = 2 * p + k
                nc.tensor.matmul(
                    bank(YB[p], slice(32 * k, 32 * k + 32)),
                    WoutT, hnxt[:, c * C : (c + 1) * C],
                    start=True, stop=True,
                    skip_group_check=(k == 1),
                )
            dst = y_all[0:64, t * 2 * C + p * C : t * 2 * C + (p + 1) * C]
            eng = YC
            if eng == "split":
                eng = "scalar" if p == 0 else "vector"
            if eng == "scalar":
                nc.scalar.activation(dst, bank(YB[p], slice(0, 64)), Act.Copy)
            else:
                nc.vector.tensor_scalar_add(dst, bank(YB[p], slice(0, 64)),
                                            b_y[0:64])
            if p == 1 and t > 0 and t % DMA_EVERY == 0:
                lo = (t - DMA_EVERY) * 2 * C
                nc.sync.dma_start(out=dout[:, lo : t * 2 * C],
                                  in_=y_all[:, lo : t * 2 * C])

        def pe_warmup(n):
            for i in range(n):
                nc.tensor.matmul(bank(7), A_r, scst[:, 0:C],
                                 start=True, stop=True, skip_group_check=True)

        def body():
            pe_warmup(WARMUP_MM)
            ph1(0, 0)
            ph1(1, 0)
            for t in range(STEPS):
                ph2(0, t)
                if t + 1 < STEPS:
                    ph1(0, t + 1)
                ph2(1, t)
                if t + 1 < STEPS:
                    ph1(1, t + 1)
            lo = ((STEPS - 1) // DMA_EVERY) * DMA_EVERY * 2 * C
            nc.sync.dma_start(out=dout[:, lo : STEPS * 2 * C],
                              in_=y_all[:, lo : STEPS * 2 * C])

        if repeats == 1:
            body()
        else:
            pe_warmup(WARMUP_ONCE)
            with tc.For_i(0, repeats, 1, staggered_reset=True):
                nc.sync.dma_start(out=hT[:], in_=dh[:, :])
                body()

    return nc


def _host_prep(x, h, W_ih, W_hh, b_ih, b_hh, W_out, b_out):
    """Fold weights on the host (float64 for exactness), build per-core maps."""
    x = np.asarray(x, dtype=np.float32)
    h = np.asarray(h, dtype=np.float32)
    W_ih = np.asarray(W_ih, dtype=np.float64)
    W_hh = np.asarray(W_hh, dtype=np.float64)
    b_ih = np.asarray(b_ih, dtype=np.float64)
    b_hh = np.asarray(b_hh, dtype=np.float64)
    W_out = np.asarray(W_out, dtype=np.float64)
    b_out = np.asarray(b_out, dtype=np.float64)

    W_ih_eff = W_ih @ W_out  # [3H, H]
    b_ih_eff = W_ih @ b_out + b_ih  # [3H]

    def cvt(a):
        return np.ascontiguousarray(a, dtype=np.float16)

    CST = cvt(
        np.concatenate(
            [
                (W_hh[0:H] + W_ih_eff[0:H]).T,       # A_r
                (W_hh[H:2*H] + W_ih_eff[H:2*H]).T,   # A_z
                W_hh[2*H:3*H].T,                      # A_hn
                W_ih_eff[2*H:3*H].T,                  # A_in
                W_hh[0:H].T,                          # A0_r
                W_hh[H:2*H].T,                        # A0_z
                W_out.T,                              # WoutT
            ],
            axis=1,
        )
    )  # [H, CW]
    W0 = cvt(np.concatenate(
        [W_ih[0:H].T, W_ih[H:2*H].T, W_ih[2*H:3*H].T], axis=1))

    by = np.zeros(H)  # b_out folded on host (ACT Copy path has no bias)
    BIAS = np.ascontiguousarray(
        np.stack(
            [
                b_hh[0:H] + b_ih_eff[0:H],
                b_hh[H:2*H] + b_ih_eff[H:2*H],
                b_hh[2*H:3*H],
                b_ih_eff[2*H:3*H],
                b_hh[0:H] + b_ih[0:H],
                b_hh[H:2*H] + b_ih[H:2*H],
                b_ih[2*H:3*H],
                by,
            ],
            axis=1,
        ),
        dtype=np.float32,
    )  # [H, 8]

    x0T = x[:, SEQLEN, :].T.astype(np.float16)  # [I, B]
    h0T = h[0].T.astype(np.float16)  # [H, B]

    in_maps = []
    for core in range(NCORES):
        cs = slice(core * BC, (core + 1) * BC)
        in_maps.append(
            {
                "cst": CST,
                "w0": W0,
                "x0": np.ascontiguousarray(x0T[:, cs]),
                "h0t": np.ascontiguousarray(h0T[:, cs]),
                "bias": BIAS,
            }
        )
    return in_maps


def _unshuffle(out_dev):
    """[64, STEPS*2C] device layout -> [BC, STEPS, I]."""
    v = out_dev.reshape(2, I, STEPS, 2, C)  # [ph, i, t, b, q]
    return np.ascontiguousarray(
        v.transpose(3, 0, 4, 2, 1).reshape(BC, STEPS, I)
    )


def _get_nc(repeats=1):
    key = (repeats, ZBAR_ENG, M2_ENG, YC, WARMUP_MM, WARMUP_ONCE, DMA_EVERY, GFUSE)
    if key not in _CACHE:
        nc = _build(repeats)
        nc.finalize()
        _CACHE[key] = nc
    return _CACHE[key]


def run(in_maps, repeats=1):
    global LAST_RESULT
    from concourse.bass_utils import run_bass_kernel_spmd

    nc = _get_nc(repeats)
    res = run_bass_kernel_spmd(nc, in_maps, core_ids=list(range(NCORES)))
    LAST_RESULT = res
    return res


def gather(res):
    return np.concatenate([_unshuffle(r["out"]) for r in res.results], axis=0)


def kernel(x, h, W_ih, W_hh, b_ih, b_hh, W_out, b_out):
    in_maps = _host_prep(x, h, W_ih, W_hh, b_ih, b_hh, W_out, b_out)
    res = run(in_maps, repeats=1)
    out = gather(res)
    out += np.asarray(b_out, dtype=np.float32)[None, None, :]
    return out


# revision 4
# speedup vs baseline: 1.0452x; 1.0452x over previous
"""Trainium2 Bass kernel for the autoregressive GRU decoder (v9).

Design (HW-measured; see memory/trn2-measured-op-costs.md):
  - pure data parallel over batch: 8 cores x 2048 rows, no collectives;
    weights folded on host (linear y-feedback absorbed into H->H gate
    matrices), fp16 state/weights, fp32 PSUM.
  - two batch-pair pipelines (2x 1024) run HALF A STEP out of phase;
    emission order is tuned for the in-order per-engine queues:
    both tanhs issue back-to-back (a ready tanh must not queue behind
    next-step sigmoids on ACT), and u/m2/zbar issue after u on DVE.
  - u = (ghn+b_hn)*r is written by DVE IN-PLACE over ghn in PSUM and
    the in-gate matmul accumulates on top with start=False (kills the
    identity-matmul accumulate, PE 24->20 matmuls/step).
  - zbar runs in DVE 4x tensor_scalar mode; Pool is unused (its SBUF
    port is shared with DVE and every Pool offload measured slower).
  - y copies on DVE; b_out folded on the host after gather.

v6 + half-step SKEWED pair pipelines: the two batch-pair pipelines are
emitted phase-shifted by half a step (ph2(p0,t), ph1(p0,t+1), ph2(p1,t),
ph1(p1,t+1), ...), so each in-order engine queue alternates between the
two pairs' phases and arrivals spread evenly — no convoy stalls at step
boundaries.

ph1(p,t): r/z-mms, sigmoids, zbar(Pool), hn-mms, u (DVE, in-place over
          ghn in PSUM), m2, in-mms (accumulate over u, start=False).
ph2(p,t): tanh, g, h', y-mms (PE), ycopy.

PSUM bank map as v6:
  b0: gr0 -> ghn0 -> u0+gin0 -> tanh    b1: chunk 1
  b4: gr2 -> ...                        b5: chunk 3
  b2: gz0                               b3: y(p0,t-1) -> gz1
  b6: gz2                               b7: y(p1,t-1) -> gz3

Sharding: pure data parallel over batch, 8 cores x 2048, no collectives.
"""

import os

import numpy as np

B, T, I, H, SEQLEN = 16384, 60, 32, 128, 30
STEPS = T - SEQLEN  # 30
NCORES = 8
BC = B // NCORES  # 2048 batch rows per core
NCH = 4
C = BC // NCH  # 512 = one PSUM bank of fp32

LAST_RESULT = None
HOST_BOUT = True
_CACHE = {}

ZBAR_ENG = os.environ.get("K_ZBAR_ENG", "vector")
M2_ENG = os.environ.get("K_M2_ENG", "vector")
YC = os.environ.get("K_YC", "vector")
DMA_EVERY = int(os.environ.get("K_DMA_EVERY", "1"))
WARMUP_MM = int(os.environ.get("K_WARMUP_MM", "0"))
WARMUP_ONCE = int(os.environ.get("K_WARMUP_ONCE", "0"))
GFUSE = int(os.environ.get("K_GFUSE", "0"))


def _build(repeats=1):
    from contextlib import ExitStack

    import concourse.bacc as bacc
    import concourse.bass as bass
    import concourse.mybir as mybir
    import concourse.tile as tile

    f32 = mybir.dt.float32
    f16 = mybir.dt.float16
    Alu = mybir.AluOpType
    Act = mybir.ActivationFunctionType

    nc = bacc.Bacc()

    CW = 6 * H + I
    dcst = nc.dram_tensor("cst", [H, CW], f16, kind="ExternalInput")
    dw0 = nc.dram_tensor("w0", [I, 3 * H], f16, kind="ExternalInput")
    dx0 = nc.dram_tensor("x0", [I, BC], f16, kind="ExternalInput")
    dh = nc.dram_tensor("h0t", [H, BC], f16, kind="ExternalInput")
    dbias = nc.dram_tensor("bias", [H, 8], f32, kind="ExternalInput")
    dout = nc.dram_tensor("out", [64, STEPS * 2 * C], f32, kind="ExternalOutput")

    with ExitStack() as ctx:
        tc = ctx.enter_context(tile.TileContext(nc))
        const = ctx.enter_context(tc.tile_pool(name="const", bufs=1))
        work = ctx.enter_context(tc.tile_pool(name="work", bufs=1))
        psum = ctx.enter_context(tc.tile_pool(name="psum", bufs=1, space="PSUM"))

        def load_const(dram, shape, name, dtype=None):
            t = const.tile(shape, dtype or dram.dtype, tag=name)
            nc.sync.dma_start(out=t[:], in_=dram[:, :])
            return t

        scst = load_const(dcst, [H, CW], "cst")
        sw0 = load_const(dw0, [I, 3 * H], "w0")
        sx0 = load_const(dx0, [I, BC], "x0")
        sbias = load_const(dbias, [H, 8], "bias")
        hT = load_const(dh, [H, BC], "h")

        A_r = scst[:, 0 * H : 1 * H]
        A_z = scst[:, 1 * H : 2 * H]
        A_hn = scst[:, 2 * H : 3 * H]
        A_in = scst[:, 3 * H : 4 * H]
        A0_r = scst[:, 4 * H : 5 * H]
        A0_z = scst[:, 5 * H : 6 * H]
        WoutT = scst[:, 6 * H : 6 * H + I]
        W0_r = sw0[:, 0 * H : 1 * H]
        W0_z = sw0[:, 1 * H : 2 * H]
        W0_n = sw0[:, 2 * H : 3 * H]

        b_r = sbias[:, 0:1]
        b_z = sbias[:, 1:2]
        b_hn = sbias[:, 2:3]
        b_in = sbias[:, 3:4]
        b0_r = sbias[:, 4:5]
        b0_z = sbias[:, 5:6]
        b0_in = sbias[:, 6:7]
        b_y = sbias[:, 7:8]

        names = ["r", "z", "n", "w", "g", "m2"]
        wt = {}
        for nm in names:
            for par in range(2):
                wt[nm, par] = work.tile(
                    [H, BC], f16, tag=f"{nm}{par}", name=f"{nm}{par}"
                )
        h2 = work.tile([H, BC], f16, tag="h2")
        h_bufs = [hT, h2]
        y_all = work.tile([64, STEPS * 2 * C], f32, tag="y")

        P = psum.tile([128, 8 * C], f32, tag="P", bufs=1)

        RB = [0, 1, 4, 5]
        NB = [2, 3, 6, 7]
        YB = [3, 7]

        def bank(b, parts=slice(0, 128)):
            return P[parts, b * C : (b + 1) * C]

        def pairb(b0):
            return P[:, b0 * C : (b0 + 2) * C]

        def x0mov(c):
            return sx0[:, c * C : (c + 1) * C]

        def half(tile_, p):
            return tile_[:, p * 2 * C : (p + 1) * 2 * C]

        def ph1(p, t):
            """r/z-mms, sigmoids, zbar, hn-mms, u, m2, in-mms for pair p."""
            first = t == 0
            par = t % 2
            hcur = h_bufs[par]
            r_sb, z_sb = wt["r", par], wt["z", par]
            w_sb, m2_sb = wt["w", par], wt["m2", par]

            def hmov(c):
                return hcur[:, c * C : (c + 1) * C]

            for c in (2 * p, 2 * p + 1):
                nc.tensor.matmul(bank(RB[c]), A0_r if first else A_r,
                                 hmov(c), start=True, stop=not first)
                if first:
                    nc.tensor.matmul(bank(RB[c]), W0_r, x0mov(c),
                                     start=False, stop=True)
            for c in (2 * p, 2 * p + 1):
                nc.tensor.matmul(bank(NB[c]), A0_z if first else A_z, hmov(c),
                                 start=True, stop=not first)
                if first:
                    nc.tensor.matmul(bank(NB[c]), W0_z, x0mov(c),
                                     start=False, stop=True)

            cb_r, cb_z = (b0_r, b0_z) if first else (b_r, b_z)
            nc.scalar.activation(half(r_sb, p), pairb(RB[2 * p]),
                                 Act.Sigmoid, bias=cb_r)
            nc.scalar.activation(half(z_sb, p), pairb(NB[2 * p]),
                                 Act.Sigmoid, bias=cb_z)
            if not GFUSE and ZBAR_ENG != "vector":
                getattr(nc, ZBAR_ENG).tensor_scalar(
                    half(w_sb, p), half(z_sb, p), -1.0, 1.0,
                    Alu.mult, Alu.add)

            for c in (2 * p, 2 * p + 1):
                nc.tensor.matmul(bank(RB[c]), A_hn, hmov(c),
                                 start=True, stop=True)

            nc.vector.scalar_tensor_tensor(
                pairb(RB[2 * p]), pairb(RB[2 * p]), b_hn, half(r_sb, p),
                Alu.add, Alu.mult)
            m2e = M2_ENG
            if m2e == "split":
                m2e = "gpsimd" if p == 0 else "vector"
            getattr(nc, m2e).tensor_tensor(
                half(m2_sb, p), half(z_sb, p), half(hcur, p), Alu.mult)
            if not GFUSE and ZBAR_ENG == "vector":
                nc.vector.tensor_scalar(
                    half(w_sb, p), half(z_sb, p), -1.0, 1.0,
                    Alu.mult, Alu.add)
            for c in (2 * p, 2 * p + 1):
                nc.tensor.matmul(bank(RB[c]),
                                 W0_n if first else A_in,
                                 x0mov(c) if first else hmov(c),
                                 start=False, stop=True,
                                 skip_group_check=True)

        def ph2a(p, t):
            """tanh only — emitted for BOTH pairs back-to-back so the
            ready tanh of pair 1 is not queued behind next-step sigmoids
            on the in-order ACT queue."""
            first = t == 0
            par = t % 2
            n_sb = wt["n", par]
            cb_in = b0_in if first else b_in
            nc.scalar.activation(half(n_sb, p), pairb(RB[2 * p]),
                                 Act.Tanh, bias=cb_in)

        def ph2b(p, t):
            """g, h', y-mms, ycopy for pair p."""
            first = t == 0
            par = t % 2
            hcur, hnxt = h_bufs[par], h_bufs[1 - par]
            n_sb = wt["n", par]
            w_sb, g_sb, m2_sb = wt["w", par], wt["g", par], wt["m2", par]
            if GFUSE:
                # g' = (z - 1) * n ; h' = m2 - g'  (no zbar op needed)
                nc.vector.scalar_tensor_tensor(
                    half(g_sb, p), half(wt["z", t % 2], p), -1.0,
                    half(n_sb, p), Alu.add, Alu.mult)
                nc.vector.tensor_tensor(half(hnxt, p), half(m2_sb, p),
                                        half(g_sb, p), Alu.subtract)
            else:
                nc.vector.tensor_tensor(half(g_sb, p), half(w_sb, p),
                                        half(n_sb, p), Alu.mult)
                nc.vector.tensor_tensor(half(hnxt, p), half(g_sb, p),
                                        half(m2_sb, p), Alu.add)

            # y(t) for pair p: reads h'(t) = hnxt
            for k in range(2):
                c = 2 * p + k
                nc.tensor.matmul(
                    bank(YB[p], slice(32 * k, 32 * k + 32)),
                    WoutT, hnxt[:, c * C : (c + 1) * C],
                    start=True, stop=True,
                    skip_group_check=(k == 1),
                )
            dst = y_all[0:64, t * 2 * C + p * C : t * 2 * C + (p + 1) * C]
            eng = YC
            if eng == "split":
                eng = "scalar" if p == 0 else "vector"
            if eng == "scalar":
                nc.scalar.activation(dst, bank(YB[p], slice(0, 64)), Act.Copy)
            else:
                nc.vector.tensor_scalar_add(dst, bank(YB[p], slice(0, 64)),
                                            b_y[0:64])
            if p == 1 and t > 0 and t % DMA_EVERY == 0:
                lo = (t - DMA_EVERY) * 2 * C
                nc.sync.dma_start(out=dout[:, lo : t * 2 * C],
                                  in_=y_all[:, lo : t * 2 * C])

        def pe_warmup(n):
            for i in range(n):
                nc.tensor.matmul(bank(7), A_r, scst[:, 0:C],
                                 start=True, stop=True, skip_group_check=True)

        def body():
            pe_warmup(WARMUP_MM)
            ph1(0, 0)
            ph1(1, 0)
            for t in range(STEPS):
                ph2a(0, t)
                ph2a(1, t)
                ph2b(0, t)
                if t + 1 < STEPS:
                    ph1(0, t + 1)
                ph2b(1, t)
                if t + 1 < STEPS:
                    ph1(1, t + 1)
            lo = ((STEPS - 1) // DMA_EVERY) * DMA_EVERY * 2 * C
            nc.sync.dma_start(out=dout[:, lo : STEPS * 2 * C],
                              in_=y_all[:, lo : STEPS * 2 * C])

        if repeats == 1:
            body()
        else:
            pe_warmup(WARMUP_ONCE)
            with tc.For_i(0, repeats, 1, staggered_reset=True):
                nc.sync.dma_start(out=hT[:], in_=dh[:, :])
                body()

    return nc


def _host_prep(x, h, W_ih, W_hh, b_ih, b_hh, W_out, b_out):
    """Fold weights on the host (float64 for exactness), build per-core maps."""
    x = np.asarray(x, dtype=np.float32)
    h = np.asarray(h, dtype=np.float32)
    W_ih = np.asarray(W_ih, dtype=np.float64)
    W_hh = np.asarray(W_hh, dtype=np.float64)
    b_ih = np.asarray(b_ih, dtype=np.float64)
    b_hh = np.asarray(b_hh, dtype=np.float64)
    W_out = np.asarray(W_out, dtype=np.float64)
    b_out = np.asarray(b_out, dtype=np.float64)

    W_ih_eff = W_ih @ W_out  # [3H, H]
    b_ih_eff = W_ih @ b_out + b_ih  # [3H]

    def cvt(a):
        return np.ascontiguousarray(a, dtype=np.float16)

    CST = cvt(
        np.concatenate(
            [
                (W_hh[0:H] + W_ih_eff[0:H]).T,       # A_r
                (W_hh[H:2*H] + W_ih_eff[H:2*H]).T,   # A_z
                W_hh[2*H:3*H].T,                      # A_hn
                W_ih_eff[2*H:3*H].T,                  # A_in
                W_hh[0:H].T,                          # A0_r
                W_hh[H:2*H].T,                        # A0_z
                W_out.T,                              # WoutT
            ],
            axis=1,
        )
    )  # [H, CW]
    W0 = cvt(np.concatenate(
        [W_ih[0:H].T, W_ih[H:2*H].T, W_ih[2*H:3*H].T], axis=1))

    by = np.zeros(H)  # b_out folded on host (ACT Copy path has no bias)
    BIAS = np.ascontiguousarray(
        np.stack(
            [
                b_hh[0:H] + b_ih_eff[0:H],
                b_hh[H:2*H] + b_ih_eff[H:2*H],
                b_hh[2*H:3*H],
                b_ih_eff[2*H:3*H],
                b_hh[0:H] + b_ih[0:H],
                b_hh[H:2*H] + b_ih[H:2*H],
                b_ih[2*H:3*H],
                by,
            ],
            axis=1,
        ),
        dtype=np.float32,
    )  # [H, 8]

    x0T = x[:, SEQLEN, :].T.astype(np.float16)  # [I, B]
    h0T = h[0].T.astype(np.float16)  # [H, B]

    in_maps = []
    for core in range(NCORES):
        cs = slice(core * BC, (core + 1) * BC)
        in_maps.append(
            {
                "cst": CST,
                "w0": W0,
                "x0": np.ascontiguousarray(x0T[:, cs]),
                "h0t": np.ascontiguousarray(h0T[:, cs]),
                "bias": BIAS,
            }
        )
    return in_maps


def _unshuffle(out_dev):
    """[64, STEPS*2C] device layout -> [BC, STEPS, I]."""
    v = out_dev.reshape(2, I, STEPS, 2, C)  # [ph, i, t, b, q]
    return np.ascontiguousarray(
        v.transpose(3, 0, 4, 2, 1).reshape(BC, STEPS, I)
    )


def _get_nc(repeats=1):
    key = (repeats, ZBAR_ENG, M2_ENG, YC, WARMUP_MM, WARMUP_ONCE, DMA_EVERY, GFUSE)
    if key not in _CACHE:
        nc = _build(repeats)
        nc.finalize()
        _CACHE[key] = nc
    return _CACHE[key]


def run(in_maps, repeats=1):
    global LAST_RESULT
    from concourse.bass_utils import run_bass_kernel_spmd

    nc = _get_nc(repeats)
    res = run_bass_kernel_spmd(nc, in_maps, core_ids=list(range(NCORES)))
    LAST_RESULT = res
    return res


def gather(res):
    return np.concatenate([_unshuffle(r["out"]) for r in res.results], axis=0)


def kernel(x, h, W_ih, W_hh, b_ih, b_hh, W_out, b_out):
    in_maps = _host_prep(x, h, W_ih, W_hh, b_ih, b_hh, W_out, b_out)
    res = run(in_maps, repeats=1)
    out = gather(res)
    out += np.asarray(b_out, dtype=np.float32)[None, None, :]
    return out
